# revision 12
# baseline (speedup 1.0000x reference)
"""Trainium2 Bass kernel for nn_AdjacencyMatrix (gnn_message_passing).

Reference computation:
    m = pad(x, [N, 1024]); repeat num_steps: m = 0.9 * (m @ W)
    y = m[:, -128:] * diag(W)[-128:]

Key algebraic collapse: only the first 256 columns of the padded state are
nonzero and only the last 128 output columns are read, so

    y = 0.9^k * x @ B,   B = (W^k)[0:256, -128:] * diag(W)[-128:]   (per col)

B is computed on-chip via the transposed chain T_i = ((W^i)[0:256, :]).T,
which uses W directly as the matmul stationary operand (no W transpose):

    T_1 = (W[0:256, :]).T            (16 PE tile transposes)
    T_{i+1} = W.T @ T_i              (f32r matmuls, 256-wide moving operand)
    T_k only needs row-tile 7 (cols 896:1024 of W^k)
    B = (diag-scaled T_k).T          (2 PE tile transposes, bf16)

Final: y = x @ B via PE with x transposed on-chip; the y-phase runs in bf16
(f32r matmuls with a 128-wide moving dim run at 1/4 rate; bf16 is full rate).

Sharding: data-parallel over the batch dim N=16384 across 8 cores (2048 rows
per core); W replicated; no collectives.

Schedule notes:
  - W is loaded as 8 single row-tile DMAs split across the two HWDGE queues
    (sync + scalar), order-chained only (no completion waits: the ring FIFO
    already serializes bytes within a queue). x follows W on each queue.
  - The first chain step consumes W row-tiles a-major in arrival order, with
    four 2-tile-wide PSUM accumulators live at once, so the step completes
    ~1us after the last W byte instead of replaying groups serially.
  - The ACT engine runs no compute (evictions alternate DVE <-> Pool), which
    removes the ACT_TABLE_LOAD from the scalar queue's path and lets its W
    DMAs start earlier.
  - diag(W)[-128:] is extracted on-chip (mask with identity + ones-matmul)
    instead of a 128-descriptor DMA gather.
  - A burst of dummy PE transposes at t=0 trips the PE_HAM activity monitor
    so the real matmuls run at full clock; small fill bursts keep it warm
    across the DMA-starved first chain step.
"""
import numpy as np

import concourse.bass as bass
import concourse.tile as tile
from concourse import bacc, mybir
from concourse.bass import _add_dep_helper
from concourse.bass_utils import run_bass_kernel_spmd
from concourse.masks import make_identity

F32 = mybir.dt.float32
F32R = mybir.dt.float32r
BF16 = mybir.dt.bfloat16

P = 128
N_ROWS = 16384
N_CORES = 8
ROWS_PER_CORE = N_ROWS // N_CORES  # 2048
D_IN = 256
N_NEURONS = 1024
N_OUT = 128
ENERGY_SCALAR = 0.9

NT = N_NEURONS // P  # 8 row/col tiles of W
DT = D_IN // P  # 2
BT = ROWS_PER_CORE // P  # 16 batch tiles per core

N_WARMUP = 28  # dummy PE ops to trip the HAM clock gate


def build(num_steps: int) -> "bacc.Bacc":
    assert num_steps >= 1
    nc = bacc.Bacc("TRN2", target_bir_lowering=False, debug=False)

    x_d = nc.dram_tensor("x", [ROWS_PER_CORE, D_IN], F32R, kind="ExternalInput").ap()
    w_d = nc.dram_tensor(
        "weight", [N_NEURONS, N_NEURONS], F32R, kind="ExternalInput"
    ).ap()
    out_d = nc.dram_tensor(
        "out", [ROWS_PER_CORE, N_OUT], F32, kind="ExternalOutput"
    ).ap()

    # alternate PSUM evictions across DVE and ACT (GPSIMD cannot touch PSUM)
    _ev = [0]

    def evict(out_ap, in_ap):
        _ev[0] += 1
        if _ev[0] % 2:
            nc.vector.tensor_copy(out_ap, in_ap)
        else:
            nc.scalar.copy(out_ap, in_ap)

    with tile.TileContext(nc) as tc:
        with tc.tile_pool(name="persist", bufs=1) as pp, \
             tc.tile_pool(name="ps_pool", bufs=8, space="PSUM") as ps_pool:
            # identity first: it gates every PE transpose (incl. warm-up)
            ident_f = pp.tile([P, P], F32)
            make_identity(nc, ident_f)
            ident_r = pp.tile([P, P], F32R)
            nc.vector.tensor_copy(ident_r[:], ident_f[:])
            ident_b = pp.tile([P, P], BF16)
            nc.gpsimd.tensor_copy(ident_b[:], ident_f[:])
            ones_f = pp.tile([P, 2], F32)
            nc.gpsimd.memset(ones_f[:], 1.0)
            ones_r = pp.tile([P, 2], F32R)
            nc.vector.tensor_copy(ones_r[:], ones_f[:])

            # ---- PE warm-up: dummy transposes, results never read ----
            def dummy_burst(n):
                for _ in range(n // 4):
                    ps = ps_pool.tile([P, 4, N_OUT], F32R, tag="ps")
                    for i in range(4):
                        nc.tensor.transpose(ps[:, i, :], ident_r[:], ident_r[:])

            dummy_burst(N_WARMUP)

            # ---- input DMAs (HWDGE, f32r end-to-end) ----
            # W as 8 single row-tile transfers: even tiles on the sync queue,
            # odd on the scalar queue; x follows W on each queue. Deps are
            # order-only (sync=False): each HW queue's ring FIFO serializes
            # the bytes, so completion waits would only idle the queue.
            w_sb = pp.tile([P, NT, N_NEURONS], F32R)
            x_sb = pp.tile([P, BT, D_IN], F32R)

            def w_load(eng, h):
                return eng.dma_start(out=w_sb[:, h, :], in_=w_d[P * h : P * (h + 1), :])

            # x in batch-major-per-partition layout: x_sb[p, t, :] =
            # x[16p + t, :], so each partition holds 16 contiguous DRAM rows
            # (the store mirrors it with 2KB descriptors). Loaded as eight
            # 4-rows-per-partition slices: 4KB descriptors, like W.
            def x_load(eng, q):
                return eng.dma_start(
                    out=x_sb[:, 4 * q : 4 * q + 4, :],
                    in_=x_d.rearrange("(p t) d -> p t d", p=P)[
                        :, 4 * q : 4 * q + 4, :
                    ],
                )

            # W tiles are order-chained only (they interleave harmlessly on
            # the queue), but x must completion-wait the queue's LAST W tile:
            # the HW queue round-robins descriptors across outstanding jobs,
            # so an early x issue steals W bandwidth.
            chains = [
                [w_load(nc.sync, h) for h in (0, 2, 4, 6)]
                + [x_load(nc.sync, q) for q in (0, 1)],
                [w_load(nc.scalar, h) for h in (1, 3, 5, 7)]
                + [x_load(nc.scalar, q) for q in (2, 3)],
            ]
            for chain in chains:
                for n, (a, b) in enumerate(zip(chain[1:], chain[:-1])):
                    _add_dep_helper(
                        a.ins, b.ins, sync=(n == 3), reason="load order"
                    )

            # W row-tile arrival order (queues alternate)
            arrival = [0, 1, 2, 3, 4, 5, 6, 7]

            # ---- diag(W)[-128:], on-chip: mask tile-7 tail with identity,
            # column-reduce via a 2-wide matmul; scaled by 0.9^k. Emitted
            # after the chain matmuls (it needs W row-tile 7, which lands
            # last) so its PSUM alloc reuses a freed chain bank.
            diag_sc = pp.tile([P, 1], F32)
            masked = pp.tile([P, N_OUT], F32R)

            def make_diag():
                nc.vector.tensor_mul(masked[:], w_sb[:, 7, 896:1024], ident_r[:])
                ps_d = ps_pool.tile([P, 2], F32, tag="ps")
                nc.tensor.matmul(
                    ps_d[:], lhsT=masked[:], rhs=ones_r[:], start=True, stop=True
                )
                nc.vector.tensor_scalar_mul(
                    diag_sc[:], ps_d[:, 0:1], float(ENERGY_SCALAR**num_steps)
                )

            # ---- T_1 = (W[0:256, :]).T : [128, 8, 256] (w tiles 0,1 only) ----
            def make_T1(j_tiles, dst):
                for gi in range(0, len(j_tiles), 2):  # 2 j-tiles per bank
                    js = j_tiles[gi : gi + 2]
                    ps = ps_pool.tile([P, DT, DT, P], F32R, tag="ps")
                    for jj, j in enumerate(js):
                        for t in range(DT):
                            nc.tensor.transpose(
                                ps[:, jj, t, :],
                                w_sb[:, t, P * j : P * (j + 1)],
                                ident_r[:],
                            )
                    evict(
                        dst[:, gi : gi + len(js), :].rearrange(
                            "p j (t c) -> p j t c", t=DT
                        ),
                        ps[:, : len(js), :, :],
                    )

            # ---- chain steps ----
            # PSUM accumulation-group rule (observed on HW): start=True
            # resets the whole bank's has-written bitmap (data survives), so
            # two groups may share a bank only SEQUENTIALLY, never
            # interleaved in time.
            #
            # streaming step (first multiply): consume W row-tiles a-major in
            # arrival order with all 8 j-tiles accumulating at once, one
            # bank each: the step finishes ~1us after the last W byte.
            def chain_step_streaming(src, dst):
                tiles = [
                    ps_pool.tile([P, D_IN], F32, tag="ps", name=f"mm{i}")
                    for i in range(NT)
                ]
                for an, a in enumerate(arrival):
                    for j in range(NT):
                        nc.tensor.matmul(
                            tiles[j][:],
                            lhsT=w_sb[:, a, P * j : P * (j + 1)],
                            rhs=src[:, a, :],
                            start=(an == 0),
                            stop=(an == NT - 1),
                        )
                for j in range(NT):
                    # DVE only: ACT is issuing the x DMAs at this moment, and
                    # an in-order ACT eviction here would stall T3's start
                    nc.vector.tensor_copy(dst[:, j, :], tiles[j][:])

            # resident step: all inputs in SBUF. Two j-tiles share a bank as
            # back-to-back sequential accumulation groups; one wide evict.
            def chain_step_resident(src, dst, after_pair=None):
                for g in range(4):
                    ps = ps_pool.tile([P, DT, D_IN], F32, tag="ps")
                    for h in range(2):
                        for an, a in enumerate(range(NT)):
                            nc.tensor.matmul(
                                ps[:, h, :],
                                lhsT=w_sb[:, a, P * (2 * g + h) : P * (2 * g + h + 1)],
                                rhs=src[:, a, :],
                                start=(an == 0),
                                stop=(an == NT - 1),
                            )
                    evict(dst[:, 2 * g : 2 * g + 2, :], ps[:, :, :])
                    if after_pair is not None:
                        after_pair(g)

            # partial last step: only j = 7 (cols 896:1024 of W^k), scaled by
            # diag * 0.9^k on eviction, cast to bf16. May be streaming
            # (arrival-ordered) when it directly follows T1.
            def chain_partial(src, dst_bf16, a_iter):
                ps = ps_pool.tile([P, D_IN], F32, tag="ps")
                for an, a in enumerate(a_iter):
                    nc.tensor.matmul(
                        ps[:],
                        lhsT=w_sb[:, a, P * (NT - 1) : P * NT],
                        rhs=src[:, a, :],
                        start=(an == 0),
                        stop=(an == NT - 1),
                    )
                nc.vector.tensor_scalar_mul(dst_bf16[:], ps[:], diag_sc[:])

            # ---- x transposes, woven into the last resident chain step so
            # their PSUM evictions hide under chain matmuls ----
            xT = pp.tile([P, DT, ROWS_PER_CORE], BF16)
            xt_emitted = set()

            def emit_xT_group(bq):
                if bq in xt_emitted or bq >= BT // 2:
                    return
                xt_emitted.add(bq)
                ps = ps_pool.tile([P, DT, DT, P], F32R, tag="ps", name="xtp")
                for bb in range(2):
                    bt = 2 * bq + bb
                    for v in range(DT):
                        nc.tensor.transpose(
                            ps[:, v, bb, :],
                            x_sb[:, bt, P * v : P * (v + 1)],
                            ident_r[:],
                        )
                evict(
                    xT[:, :, 256 * bq : 256 * (bq + 1)].rearrange(
                        "p v (b c) -> p v b c", b=2
                    ),
                    ps[:, :, :, :],
                )

            def weave(g):
                # xT needs all x partitions; x lands mid-T3, so weave the
                # transposes into the back half of the step
                if g >= 2:
                    for k in range(4):
                        emit_xT_group(4 * (g - 2) + k)

            T4 = pp.tile([P, D_IN], BF16)  # scaled T_k row-tile 7
            if num_steps == 1:
                t1_last = pp.tile([P, 1, D_IN], F32R)
                make_T1([NT - 1], t1_last)
                make_diag()
                nc.vector.tensor_scalar_mul(T4[:], t1_last[:, 0, :], diag_sc[:])
            else:
                T_cur = pp.tile([P, NT, D_IN], F32R, name="T1")
                make_T1(list(range(NT)), T_cur)
                if num_steps == 2:
                    make_diag()
                    chain_partial(T_cur, T4, arrival)
                else:
                    T_nxt = pp.tile([P, NT, D_IN], F32R, name="T2")
                    chain_step_streaming(T_cur, T_nxt)
                    T_cur = T_nxt
                    make_diag()
                    for step in range(3, num_steps):
                        T_nxt = pp.tile([P, NT, D_IN], F32R, name=f"T{step}")
                        last_full = step == num_steps - 1
                        chain_step_resident(
                            T_cur, T_nxt, after_pair=weave if last_full else None
                        )
                        T_cur = T_nxt
                    chain_partial(T_cur, T4, range(NT))
            for bq in range(BT // 2):
                emit_xT_group(bq)

            # ---- B = (T4).T : [128, 2, 128] bf16 ----
            B_sb = pp.tile([P, DT, N_OUT], BF16)
            ps_b = ps_pool.tile([P, DT, P], BF16, tag="ps")
            for u in range(DT):
                nc.tensor.transpose(
                    ps_b[:, u, :], T4[:, P * u : P * (u + 1)], ident_b[:]
                )
            nc.vector.tensor_copy(B_sb[:, 0, :], ps_b[:, 0, :])
            nc.scalar.copy(B_sb[:, 1, :], ps_b[:, 1, :])

            # ---- y[b, c] = sum_v xT[:, v, b].T @ B[:, v, :]  (bf16) ----
            # y tile t holds batch rows {16m + t}, matching x_sb's layout, so
            # the store is contiguous per partition.
            out_r = out_d.rearrange("(p t) c -> p t c", p=P)
            y_sb = pp.tile([P, BT, N_OUT], F32)
            for g in range(4):
                ps = ps_pool.tile([P, 4, N_OUT], F32, tag="ps")
                for i in range(4):
                    bt = 4 * g + i
                    for v in range(DT):
                        nc.tensor.matmul(
                            ps[:, i, :],
                            lhsT=xT[:, v, P * bt : P * (bt + 1)],
                            rhs=B_sb[:, v, :],
                            start=(v == 0),
                            stop=(v == DT - 1),
                        )
                # split each eviction across both engines so the store can
                # issue as soon as possible
                nc.vector.tensor_copy(y_sb[:, 4 * g : 4 * g + 2, :], ps[:, 0:2, :])
                nc.scalar.copy(y_sb[:, 4 * g + 2 : 4 * g + 4, :], ps[:, 2:4, :])
                if g == 3:
                    for h in range(2):
                        oeng = nc.sync if h == 0 else nc.scalar
                        oeng.dma_start(
                            out=out_r[:, 4 * g + 2 * h : 4 * g + 2 * h + 2, :],
                            in_=y_sb[:, 4 * g + 2 * h : 4 * g + 2 * h + 2, :],
                        )
                else:
                    oeng = nc.sync if g % 2 == 0 else nc.scalar
                    oeng.dma_start(
                        out=out_r[:, 4 * g : 4 * g + 4, :],
                        in_=y_sb[:, 4 * g : 4 * g + 4, :],
                    )

    nc.compile()
    return nc


_NC_CACHE: dict = {}


def _get_nc(num_steps: int):
    if num_steps not in _NC_CACHE:
        _NC_CACHE[num_steps] = build(num_steps)
    return _NC_CACHE[num_steps]


def kernel(x: np.ndarray, weight: np.ndarray, num_steps) -> np.ndarray:
    k = int(num_steps)
    x = np.ascontiguousarray(x, dtype=np.float32)
    weight = np.ascontiguousarray(weight, dtype=np.float32)
    if k == 0:
        # pad(x)[:, -128:] is all zero (128 <= 1024 - 256)
        return np.zeros((x.shape[0], N_OUT), dtype=np.float32)

    nc = _get_nc(k)
    in_maps = [
        {
            "x": x[i * ROWS_PER_CORE : (i + 1) * ROWS_PER_CORE],
            "weight": weight,
        }
        for i in range(N_CORES)
    ]
    last_err = None
    for attempt in range(3):
        try:
            res = run_bass_kernel_spmd(nc, in_maps, core_ids=list(range(N_CORES)))
            return np.concatenate(
                [res.results[i]["out"] for i in range(N_CORES)], axis=0
            )
        except Exception as e:  # transient device wedges recover on retry
            last_err = e
            import time as _time

            _time.sleep(10)
    raise last_err


# revision 13
# speedup vs baseline: 1.0967x; 1.0967x over previous
"""Trainium2 Bass kernel for nn_AdjacencyMatrix (gnn_message_passing).

Reference computation:
    m = pad(x, [N, 1024]); repeat num_steps: m = 0.9 * (m @ W)
    y = m[:, -128:] * diag(W)[-128:]

Key algebraic collapse: only the first 256 columns of the padded state are
nonzero and only the last 128 output columns are read, so

    y = 0.9^k * x @ B,   B = (W^k)[0:256, -128:] * diag(W)[-128:]   (per col)

B is computed on-chip via the transposed chain T_i = ((W^i)[0:256, :]).T,
which uses W directly as the matmul stationary operand (no W transpose):

    T_1 = (W[0:256, :]).T            (16 PE tile transposes)
    T_{i+1} = W.T @ T_i              (f32r matmuls, 256-wide moving operand)
    T_k only needs row-tile 7 (cols 896:1024 of W^k)
    B = (diag-scaled T_k).T          (2 PE tile transposes, bf16)

Final: y = x @ B via PE with x transposed on-chip; the y-phase runs in bf16
(f32r matmuls with a 128-wide moving dim run at 1/4 rate; bf16 is full rate).

Sharding: data-parallel over the batch dim N=16384 across 8 cores (2048 rows
per core); W replicated; no collectives.

Schedule notes:
  - W is loaded as 8 single row-tile DMAs split across the two HWDGE queues
    (sync + scalar), order-chained only (no completion waits: the ring FIFO
    already serializes bytes within a queue). x follows W on each queue.
  - The first chain step consumes W row-tiles a-major in arrival order, with
    four 2-tile-wide PSUM accumulators live at once, so the step completes
    ~1us after the last W byte instead of replaying groups serially.
  - The ACT engine runs no compute (evictions alternate DVE <-> Pool), which
    removes the ACT_TABLE_LOAD from the scalar queue's path and lets its W
    DMAs start earlier.
  - diag(W)[-128:] is extracted on-chip (mask with identity + ones-matmul)
    instead of a 128-descriptor DMA gather.
  - A burst of dummy PE transposes at t=0 trips the PE_HAM activity monitor
    so the real matmuls run at full clock; small fill bursts keep it warm
    across the DMA-starved first chain step.
"""
import numpy as np

import concourse.bass as bass
import concourse.tile as tile
from concourse import bacc, mybir
from concourse.bass import _add_dep_helper
from concourse.bass_utils import run_bass_kernel_spmd
from concourse.masks import make_identity

F32 = mybir.dt.float32
F32R = mybir.dt.float32r
BF16 = mybir.dt.bfloat16

P = 128
N_ROWS = 16384
N_CORES = 8
ROWS_PER_CORE = N_ROWS // N_CORES  # 2048
D_IN = 256
N_NEURONS = 1024
N_OUT = 128
ENERGY_SCALAR = 0.9

NT = N_NEURONS // P  # 8 row/col tiles of W
DT = D_IN // P  # 2
BT = ROWS_PER_CORE // P  # 16 batch tiles per core

N_WARMUP = 28  # dummy PE ops to trip the HAM clock gate


def build(num_steps: int) -> "bacc.Bacc":
    assert num_steps >= 1
    nc = bacc.Bacc("TRN2", target_bir_lowering=False, debug=False)

    x_d = nc.dram_tensor("x", [ROWS_PER_CORE, D_IN], F32R, kind="ExternalInput").ap()
    w_d = nc.dram_tensor(
        "weight", [N_NEURONS, N_NEURONS], F32R, kind="ExternalInput"
    ).ap()
    out_d = nc.dram_tensor(
        "out", [ROWS_PER_CORE, N_OUT], F32, kind="ExternalOutput"
    ).ap()

    # alternate PSUM evictions across DVE and ACT (GPSIMD cannot touch PSUM)
    _ev = [0]

    def evict(out_ap, in_ap):
        _ev[0] += 1
        if _ev[0] % 2:
            nc.vector.tensor_copy(out_ap, in_ap)
        else:
            nc.scalar.copy(out_ap, in_ap)

    with tile.TileContext(nc) as tc:
        with tc.tile_pool(name="persist", bufs=1) as pp, \
             tc.tile_pool(name="ps_pool", bufs=8, space="PSUM") as ps_pool:
            # identity first: it gates every PE transpose (incl. warm-up)
            ident_f = pp.tile([P, P], F32)
            make_identity(nc, ident_f)
            ident_r = pp.tile([P, P], F32R)
            nc.vector.tensor_copy(ident_r[:], ident_f[:])
            ident_b = pp.tile([P, P], BF16)
            nc.gpsimd.tensor_copy(ident_b[:], ident_f[:])
            ones_f = pp.tile([P, 2], F32)
            nc.gpsimd.memset(ones_f[:], 1.0)
            ones_r = pp.tile([P, 2], F32R)
            nc.vector.tensor_copy(ones_r[:], ones_f[:])

            # ---- PE warm-up: dummy transposes, results never read ----
            def dummy_burst(n):
                for _ in range(n // 4):
                    ps = ps_pool.tile([P, 4, N_OUT], F32R, tag="ps")
                    for i in range(4):
                        nc.tensor.transpose(ps[:, i, :], ident_r[:], ident_r[:])

            dummy_burst(N_WARMUP)

            # ---- input DMAs (HWDGE, f32r end-to-end) ----
            # W as 8 single row-tile transfers: even tiles on the sync queue,
            # odd on the scalar queue; x follows W on each queue. Deps are
            # order-only (sync=False): each HW queue's ring FIFO serializes
            # the bytes, so completion waits would only idle the queue.
            w_sb = pp.tile([P, NT, N_NEURONS], F32R)
            x_sb = pp.tile([P, BT, D_IN], F32R)

            def w_load(eng, h):
                return eng.dma_start(out=w_sb[:, h, :], in_=w_d[P * h : P * (h + 1), :])

            def x_load(eng, q):
                return eng.dma_start(
                    out=x_sb[:, 4 * q : 4 * q + 4, :],
                    in_=x_d[512 * q : 512 * (q + 1), :].rearrange(
                        "(t p) d -> p t d", p=P
                    ),
                )

            chains = [
                [w_load(nc.sync, h) for h in (0, 2, 4, 6)]
                + [x_load(nc.sync, 0), x_load(nc.sync, 1)],
                [w_load(nc.scalar, h) for h in (1, 3, 5, 7)]
                + [x_load(nc.scalar, 2), x_load(nc.scalar, 3)],
            ]
            for chain in chains:
                for a, b in zip(chain[1:], chain[:-1]):
                    _add_dep_helper(a.ins, b.ins, sync=False, reason="load order")

            # W row-tile arrival order (queues alternate)
            arrival = [0, 1, 2, 3, 4, 5, 6, 7]

            # ---- diag(W)[-128:], on-chip: mask tile-7 tail with identity,
            # column-reduce via a 2-wide matmul; scaled by 0.9^k. Emitted
            # after the chain matmuls (it needs W row-tile 7, which lands
            # last) so its PSUM alloc reuses a freed chain bank.
            diag_sc = pp.tile([P, 1], F32)
            masked = pp.tile([P, N_OUT], F32R)

            def make_diag():
                nc.vector.tensor_mul(masked[:], w_sb[:, 7, 896:1024], ident_r[:])
                ps_d = ps_pool.tile([P, 2], F32, tag="ps")
                nc.tensor.matmul(
                    ps_d[:], lhsT=masked[:], rhs=ones_r[:], start=True, stop=True
                )
                nc.vector.tensor_scalar_mul(
                    diag_sc[:], ps_d[:, 0:1], float(ENERGY_SCALAR**num_steps)
                )

            # ---- T_1 = (W[0:256, :]).T : [128, 8, 256] (w tiles 0,1 only) ----
            def make_T1(j_tiles, dst):
                for gi in range(0, len(j_tiles), 2):  # 2 j-tiles per bank
                    js = j_tiles[gi : gi + 2]
                    ps = ps_pool.tile([P, DT, DT, P], F32R, tag="ps")
                    for jj, j in enumerate(js):
                        for t in range(DT):
                            nc.tensor.transpose(
                                ps[:, jj, t, :],
                                w_sb[:, t, P * j : P * (j + 1)],
                                ident_r[:],
                            )
                    evict(
                        dst[:, gi : gi + len(js), :].rearrange(
                            "p j (t c) -> p j t c", t=DT
                        ),
                        ps[:, : len(js), :, :],
                    )

            # ---- chain steps ----
            # PSUM accumulation-group rule (observed on HW): start=True
            # resets the whole bank's has-written bitmap (data survives), so
            # two groups may share a bank only SEQUENTIALLY, never
            # interleaved in time.
            #
            # streaming step (first multiply): consume W row-tiles a-major in
            # arrival order with all 8 j-tiles accumulating at once, one
            # bank each: the step finishes ~1us after the last W byte.
            def chain_step_streaming(src, dst):
                tiles = [
                    ps_pool.tile([P, D_IN], F32, tag="ps", name=f"mm{i}")
                    for i in range(NT)
                ]
                for an, a in enumerate(arrival):
                    for j in range(NT):
                        nc.tensor.matmul(
                            tiles[j][:],
                            lhsT=w_sb[:, a, P * j : P * (j + 1)],
                            rhs=src[:, a, :],
                            start=(an == 0),
                            stop=(an == NT - 1),
                        )
                for j in range(NT):
                    evict(dst[:, j, :], tiles[j][:])

            # resident step: all inputs in SBUF. Two j-tiles share a bank as
            # back-to-back sequential accumulation groups; one wide evict.
            def chain_step_resident(src, dst, after_pair=None):
                for g in range(4):
                    ps = ps_pool.tile([P, DT, D_IN], F32, tag="ps")
                    for h in range(2):
                        for an, a in enumerate(range(NT)):
                            nc.tensor.matmul(
                                ps[:, h, :],
                                lhsT=w_sb[:, a, P * (2 * g + h) : P * (2 * g + h + 1)],
                                rhs=src[:, a, :],
                                start=(an == 0),
                                stop=(an == NT - 1),
                            )
                    evict(dst[:, 2 * g : 2 * g + 2, :], ps[:, :, :])
                    if after_pair is not None:
                        after_pair(g)

            # partial last step: only j = 7 (cols 896:1024 of W^k), scaled by
            # diag * 0.9^k on eviction, cast to bf16. May be streaming
            # (arrival-ordered) when it directly follows T1.
            def chain_partial(src, dst_bf16, a_iter):
                ps = ps_pool.tile([P, D_IN], F32, tag="ps")
                for an, a in enumerate(a_iter):
                    nc.tensor.matmul(
                        ps[:],
                        lhsT=w_sb[:, a, P * (NT - 1) : P * NT],
                        rhs=src[:, a, :],
                        start=(an == 0),
                        stop=(an == NT - 1),
                    )
                nc.vector.tensor_scalar_mul(dst_bf16[:], ps[:], diag_sc[:])

            # ---- x transposes, woven into the last resident chain step so
            # their PSUM evictions hide under chain matmuls ----
            xT = pp.tile([P, DT, ROWS_PER_CORE], BF16)
            xt_emitted = set()

            def emit_xT_group(bq):
                if bq in xt_emitted or bq >= BT // 2:
                    return
                xt_emitted.add(bq)
                ps = ps_pool.tile([P, DT, DT, P], F32R, tag="ps", name="xtp")
                for bb in range(2):
                    bt = 2 * bq + bb
                    for v in range(DT):
                        nc.tensor.transpose(
                            ps[:, v, bb, :],
                            x_sb[:, bt, P * v : P * (v + 1)],
                            ident_r[:],
                        )
                evict(
                    xT[:, :, 256 * bq : 256 * (bq + 1)].rearrange(
                        "p v (b c) -> p v b c", b=2
                    ),
                    ps[:, :, :, :],
                )

            def weave(g):
                # x chunk arrival: x0 (bq 0,1) and x2 (bq 4,5) land first
                emit_xT_group(g)
                emit_xT_group(g + 4)

            T4 = pp.tile([P, D_IN], BF16)  # scaled T_k row-tile 7
            if num_steps == 1:
                t1_last = pp.tile([P, 1, D_IN], F32R)
                make_T1([NT - 1], t1_last)
                make_diag()
                nc.vector.tensor_scalar_mul(T4[:], t1_last[:, 0, :], diag_sc[:])
            else:
                T_cur = pp.tile([P, NT, D_IN], F32R, name="T1")
                make_T1(list(range(NT)), T_cur)
                if num_steps == 2:
                    make_diag()
                    chain_partial(T_cur, T4, arrival)
                else:
                    T_nxt = pp.tile([P, NT, D_IN], F32R, name="T2")
                    chain_step_streaming(T_cur, T_nxt)
                    T_cur = T_nxt
                    make_diag()
                    for step in range(3, num_steps):
                        T_nxt = pp.tile([P, NT, D_IN], F32R, name=f"T{step}")
                        last_full = step == num_steps - 1
                        chain_step_resident(
                            T_cur, T_nxt, after_pair=weave if last_full else None
                        )
                        T_cur = T_nxt
                    chain_partial(T_cur, T4, range(NT))
            for bq in range(BT // 2):
                emit_xT_group(bq)

            # ---- B = (T4).T : [128, 2, 128] bf16 ----
            B_sb = pp.tile([P, DT, N_OUT], BF16)
            ps_b = ps_pool.tile([P, DT, P], BF16, tag="ps")
            for u in range(DT):
                nc.tensor.transpose(
                    ps_b[:, u, :], T4[:, P * u : P * (u + 1)], ident_b[:]
                )
            nc.vector.tensor_copy(B_sb[:, 0, :], ps_b[:, 0, :])
            nc.scalar.copy(B_sb[:, 1, :], ps_b[:, 1, :])

            # ---- y[b, c] = sum_v xT[:, v, b].T @ B[:, v, :]  (bf16) ----
            y_sb = pp.tile([P, BT, N_OUT], F32)
            for g in range(4):
                ps = ps_pool.tile([P, 4, N_OUT], F32, tag="ps")
                for i in range(4):
                    bt = 4 * g + i
                    for v in range(DT):
                        nc.tensor.matmul(
                            ps[:, i, :],
                            lhsT=xT[:, v, P * bt : P * (bt + 1)],
                            rhs=B_sb[:, v, :],
                            start=(v == 0),
                            stop=(v == DT - 1),
                        )
                if g == 3:
                    # last group: split across both engines to shorten the
                    # kernel tail (eviction and store both halve)
                    nc.vector.tensor_copy(
                        y_sb[:, 4 * g : 4 * g + 2, :], ps[:, 0:2, :]
                    )
                    nc.scalar.copy(
                        y_sb[:, 4 * g + 2 : 4 * g + 4, :], ps[:, 2:4, :]
                    )
                    for h in range(2):
                        oeng = nc.sync if h == 0 else nc.scalar
                        lo = 512 * g + 256 * h
                        oeng.dma_start(
                            out=out_d[lo : lo + 256, :].rearrange(
                                "(t p) c -> p t c", p=P
                            ),
                            in_=y_sb[:, 4 * g + 2 * h : 4 * g + 2 * h + 2, :],
                        )
                else:
                    evict(y_sb[:, 4 * g : 4 * g + 4, :], ps[:, :, :])
                    oeng = nc.sync if g % 2 == 0 else nc.scalar
                    oeng.dma_start(
                        out=out_d[512 * g : 512 * (g + 1), :].rearrange(
                            "(t p) c -> p t c", p=P
                        ),
                        in_=y_sb[:, 4 * g : 4 * g + 4, :],
                    )

    nc.compile()
    return nc


_NC_CACHE: dict = {}


def _get_nc(num_steps: int):
    if num_steps not in _NC_CACHE:
        _NC_CACHE[num_steps] = build(num_steps)
    return _NC_CACHE[num_steps]


def kernel(x: np.ndarray, weight: np.ndarray, num_steps) -> np.ndarray:
    k = int(num_steps)
    x = np.ascontiguousarray(x, dtype=np.float32)
    weight = np.ascontiguousarray(weight, dtype=np.float32)
    if k == 0:
        # pad(x)[:, -128:] is all zero (128 <= 1024 - 256)
        return np.zeros((x.shape[0], N_OUT), dtype=np.float32)

    nc = _get_nc(k)
    in_maps = [
        {
            "x": x[i * ROWS_PER_CORE : (i + 1) * ROWS_PER_CORE],
            "weight": weight,
        }
        for i in range(N_CORES)
    ]
    last_err = None
    for attempt in range(3):
        try:
            res = run_bass_kernel_spmd(nc, in_maps, core_ids=list(range(N_CORES)))
            return np.concatenate(
                [res.results[i]["out"] for i in range(N_CORES)], axis=0
            )
        except Exception as e:  # transient device wedges recover on retry
            last_err = e
            import time as _time

            _time.sleep(10)
    raise last_err
